# revision 35
# baseline (speedup 1.0000x reference)
"""BoxConv2d Trainium2 kernel (8 NeuronCores, SPMD).

Math: the reference's integral-image + fractional box-edge interpolation
pipeline is linear in the input and separable, so per output channel
k = (c, f) it collapses to two dense 128x128 matrix products:

    out[b,k] = A_k @ x[b,c] @ B_k^T

with banded "pixel overlap" matrices
    A_k[xo, a] = clamp(xo - a + x_max_k + 1, 0, 1)
                 - clamp(xo - a + x_min_k, 0, 1)
and likewise B_k for columns.  A/B are built on the host from the tiny
(C,F) box params; the device does pure 128-contraction matmuls.

Sharding: the K = C*F = 128 output channels are split across 8 cores
(16 channels = 4 in_planes per core), so each core reads only its own
4 input planes and input reads are not duplicated chip-wide.

Device dataflow per core (all operands bfloat16, PSUM accum fp32):
  pass 1 (per b,c):   V[j, (f,xo)]  = x_bc^T A^T  (lhsT=x_bc, N=512)
  pass 2 (per c,f,h): O[yo, (bh,xo)] = B_k V      (lhsT=B_k^T, N=512)
PSUM->SBUF drains (only Vector and Scalar can read PSUM) alternate
between those engines; pass-2 drains move 1024 cols (a 2-bank PSUM
tile filled by two half-batch matmuls) per instruction.

Layout choices are driven by measured DMA behavior: descriptor
processing is the bottleneck (~6.4ns per descriptor), so every SBUF
tile is shaped for >=2KB partition lines (x in batch-pairs, at/bt in
c-pairs, output as contiguous 2KB lines), at/bt ride their own
hardware queue in parallel with x, and nothing is issued from the
GpSimd queue (measured ~2x slower).  The PE starts at half duty
(~630ns per 512-col matmul) until the HAM power controller observes
sustained activity and grants full duty (~376ns, hard-capped at
~20.5us per grant); two fp32 dummy matmuls bridge the engine-init
window so the grant lands early in the real matmul stream.

Numerics: bf16 gives l2 rel error ~3e-3 vs the fp32 reference
(budget 2e-2); the fp32 path was at the ridge of both the DMA and
(throttled) PE rooflines, bf16 halves bytes moved and PE work.
"""

import sys

if "/opt/trn_rl_repo" not in sys.path:
    sys.path.insert(0, "/opt/trn_rl_repo")

import numpy as np
import ml_dtypes

import concourse.bass as bass  # noqa: F401
import concourse.mybir as mybir
import concourse.tile as tile
from concourse import bacc
from concourse.bass_utils import run_bass_kernel_spmd

B, C, F, H, W = 8, 32, 4, 128, 128
NCORES = 8
CPC = C // NCORES   # in_planes per core
KPC = CPC * F       # output channels per core
NP = B // 2         # x batch-pairs per core

_MM_DT = mybir.dt.bfloat16
_NP_DT = ml_dtypes.bfloat16

_NC_CACHE = {}
LAST_RESULT = None


def _build_nc():
    nc = bacc.Bacc(
        "TRN2", target_bir_lowering=False, debug=False, num_devices=NCORES
    )
    # x[p, a, (b2, c, j)]: batch-pairs give 2KB DMA lines
    x_p = nc.declare_dram_parameter(
        "x", [NP, H, 2 * CPC * W], _MM_DT, isOutput=False)
    # at[cp, a, (c2, f, xo)] / bt[cp, j, (c2, f, yo)]: c-pair tiles
    at_p = nc.declare_dram_parameter(
        "at", [CPC // 2, H, 2 * F * H], _MM_DT, isOutput=False)
    bt_p = nc.declare_dram_parameter(
        "bt", [CPC // 2, W, 2 * F * W], _MM_DT, isOutput=False)
    # transposed output: outT[yo, c, f, b, xo] = out[b, c*F+f, xo, yo]
    # -> per-(c,f) DMA writes 2048 contiguous elements (2KB) per yo line
    out_p = nc.declare_dram_parameter(
        "outT", [W, CPC, F, B * H], _MM_DT, isOutput=True)

    with tile.TileContext(nc) as tc:
        with (
            tc.tile_pool(name="const", bufs=1) as cpool,
            tc.tile_pool(name="xin", bufs=NP) as xpool,
            tc.tile_pool(name="vall", bufs=2) as vpool,
            tc.tile_pool(name="osb", bufs=4) as opool,
            tc.tile_pool(name="pv", bufs=4, space="PSUM") as pvpool,
            tc.tile_pool(name="po", bufs=2, space="PSUM") as popool,
        ):
            # PSUM->SBUF drains alternate between the two engines that
            # can read PSUM
            eng_i = [0]

            def copy(dst, src):
                if eng_i[0] % 2:
                    nc.scalar.copy(dst, src)
                else:
                    nc.vector.tensor_copy(dst, src)
                eng_i[0] += 1

            # warm-up: dummy fp32r matmuls (higher switching activity than
            # bf16) start the HAM activity clock during the ~7us engine
            # init + input DMA window, so the full-duty grant lands as
            # early as possible into the real matmul stream
            dum = cpool.tile([128, F * H], mybir.dt.float32,
                             name="dum", tag="dum")
            nc.vector.memset(dum[:], 0.0)
            for i in range(3):
                d_ps = pvpool.tile([128, F * H], mybir.dt.float32,
                                   name=f"dps{i}", tag="vps")
                nc.tensor.matmul(
                    d_ps[:],
                    lhsT=dum[:, :W],
                    rhs=dum[:],
                    start=True,
                    stop=True,
                )

            # input DMAs: only SP/Activation queues (GpSimd's hardware
            # queue measured ~2x slower); priority order matches consumer
            # order: at+first x pair gate the first matmul, bt gates the
            # first pass-2 (~+5us)
            at_sb = [None] * (CPC // 2)
            bt_sb = [None] * (CPC // 2)
            x_sb = [None] * NP

            def load_x(p, eng):
                x_sb[p] = xpool.tile(
                    [128, 2 * CPC * W], _MM_DT, name=f"xsb{p}", tag="x"
                )
                eng.dma_start(x_sb[p][:], x_p[p])

            def load_at(cp, eng):
                at_sb[cp] = cpool.tile([128, 2 * F * H], _MM_DT,
                                       name=f"at{cp}", tag=f"at{cp}")
                eng.dma_start(at_sb[cp][:], at_p[cp])

            def load_bt(cp, eng):
                bt_sb[cp] = cpool.tile([128, 2 * F * W], _MM_DT,
                                       name=f"bt{cp}", tag=f"bt{cp}")
                eng.dma_start(bt_sb[cp][:], bt_p[cp])

            load_at(0, nc.sync)
            load_x(0, nc.scalar)
            load_x(1, nc.sync)
            load_bt(0, nc.scalar)
            load_x(2, nc.sync)
            load_x(3, nc.scalar)
            load_at(1, nc.sync)
            load_bt(1, nc.scalar)

            v_full = [None] * CPC

            def emit_pass1_part(c, b0, b1):
                # V[j, (f, b, xo)] for the whole batch of plane c
                if b0 == 0:
                    v_full[c] = vpool.tile([128, F * B * H], _MM_DT,
                                           name=f"vall{c}", tag="vall")
                vt = v_full[c]
                v_r = vt[:].rearrange("p (f b xo) -> p f b xo", f=F, b=B)
                for b in range(b0, b1):
                    p, hb = b // 2, b % 2
                    # V[j, (f,xo)] = sum_a x[a, j] * A_k[xo, a]
                    v_ps = pvpool.tile([128, F * H], mybir.dt.float32,
                                       name=f"vps{c}{b}", tag="vps")
                    nc.tensor.matmul(
                        v_ps[:],
                        lhsT=x_sb[p][:, (hb * CPC + c) * W:
                                     (hb * CPC + c + 1) * W],
                        rhs=at_sb[c // 2][:, (c % 2) * F * H:
                                          (c % 2 + 1) * F * H],
                        start=True,
                        stop=True,
                    )
                    # scatter the 4 f-blocks into V's (f, b, .) slots
                    copy(v_r[:, :, b, :], v_ps[:])

            HB = B * H // 2

            def emit_pass2_f(c, f, split_tail=False):
                vt = v_full[c]
                # O[yo, (b,xo)] = sum_j B_k[yo,j] * V[j, (b,xo)]
                # matmul output must stay within one PSUM bank (512 fp32
                # cols): two half-batch matmuls fill the 2-bank tile
                o_ps = popool.tile([128, B * H], mybir.dt.float32,
                                   name=f"ops{c}{f}", tag="ops")
                for h in range(2):
                    nc.tensor.matmul(
                        o_ps[:, h * HB:(h + 1) * HB],
                        lhsT=bt_sb[c // 2][:, ((c % 2) * F + f) * W:
                                           ((c % 2) * F + f + 1) * W],
                        rhs=vt[:, f * B * H + h * HB:f * B * H + (h + 1) * HB],
                        start=True,
                        stop=True,
                    )
                o_sb = opool.tile([128, B * H], _MM_DT,
                                  name=f"osb{c}{f}", tag="osb")
                if split_tail:
                    # final groups: drain halves on both engines in
                    # parallel and start each half-DMA immediately,
                    # shortening the post-last-matmul tail
                    nc.vector.tensor_copy(o_sb[:, :HB], o_ps[:, :HB])
                    nc.sync.dma_start(out_p[:, c, f, 0:HB], o_sb[:, :HB])
                    nc.scalar.copy(o_sb[:, HB:], o_ps[:, HB:])
                    nc.sync.dma_start(out_p[:, c, f, HB:], o_sb[:, HB:])
                else:
                    copy(o_sb[:], o_ps[:])
                    # contiguous 256KB DRAM write per (c,f), 2KB per line
                    nc.sync.dma_start(out_p[:, c, f], o_sb[:])

            # software pipeline: pass2(c-1) f-groups interleave 1:2
            # with pass1(c) matmuls, shifted so the first f-group of each
            # c never waits on that c's final V drains
            emit_pass1_part(0, 0, B)
            emit_pass1_part(1, 0, 2)
            for c in range(1, CPC):
                emit_pass2_f(c - 1, 0)
                emit_pass1_part(c, 2, 4)
                emit_pass2_f(c - 1, 1)
                emit_pass1_part(c, 4, 6)
                emit_pass2_f(c - 1, 2)
                emit_pass1_part(c, 6, 8)
                emit_pass2_f(c - 1, 3)
                if c + 1 < CPC:
                    emit_pass1_part(c + 1, 0, 2)
            emit_pass2_f(CPC - 1, 0)
            emit_pass2_f(CPC - 1, 1)
            emit_pass2_f(CPC - 1, 2, split_tail=True)
            emit_pass2_f(CPC - 1, 3, split_tail=True)
    nc.finalize()
    return nc


def _get_nc():
    if "nc" not in _NC_CACHE:
        _NC_CACHE["nc"] = _build_nc()
    return _NC_CACHE["nc"]


def _overlap_mats(lo, hi):
    """(K, out, in) pixel-overlap matrices for a 128-wide axis."""
    t = np.arange(128, dtype=np.float64)
    d = t[:, None] - t[None, :]  # out - in
    lo = lo.astype(np.float64)[:, None, None]
    hi = hi.astype(np.float64)[:, None, None]
    m = np.clip(d[None] + hi + 1.0, 0.0, 1.0) - np.clip(d[None] + lo, 0.0, 1.0)
    return m.astype(np.float32)


def _make_in_maps(input, x_min, x_max, y_min, y_max):
    A = _overlap_mats(x_min.reshape(-1), x_max.reshape(-1))   # (K, xo, a)
    Bm = _overlap_mats(y_min.reshape(-1), y_max.reshape(-1))  # (K, yo, j)
    in_maps = []
    for m in range(NCORES):
        cs = slice(CPC * m, CPC * (m + 1))
        ks = slice(KPC * m, KPC * (m + 1))
        # x[p, a, (b2, c, j)]
        xm = input[:, cs]                                   # [b, c, a, j]
        xm = xm.reshape(NP, 2, CPC, H, W)
        xm = xm.transpose(0, 3, 1, 2, 4).reshape(NP, H, 2 * CPC * W)
        # at[cp, a, (c2, f, xo)] = A[k=(cp*2+c2)*F+f, xo, a]
        at = A[ks].reshape(CPC // 2, 2, F, H, H).transpose(0, 4, 1, 2, 3)
        bt = Bm[ks].reshape(CPC // 2, 2, F, W, W).transpose(0, 4, 1, 2, 3)
        in_maps.append({
            "x": np.ascontiguousarray(xm).astype(_NP_DT),
            "at": np.ascontiguousarray(
                at.reshape(CPC // 2, H, 2 * F * H)).astype(_NP_DT),
            "bt": np.ascontiguousarray(
                bt.reshape(CPC // 2, W, 2 * F * W)).astype(_NP_DT),
        })
    return in_maps


def _assemble(results):
    out = np.empty((B, C * F, H, W), np.float32)
    for m in range(NCORES):
        # outT[yo, c, f, b, xo] -> out[b, (c,f), xo, yo]
        o = results[m]["outT"].astype(np.float32).reshape(W, CPC, F, B, H)
        o = o.transpose(3, 1, 2, 4, 0).reshape(B, KPC, H, W)
        out[:, KPC * m:KPC * (m + 1)] = o
    return out


def _run(inputs, trace=False):
    global LAST_RESULT
    nc = _get_nc()
    in_maps = _make_in_maps(**inputs)
    LAST_RESULT = run_bass_kernel_spmd(
        nc, in_maps, list(range(NCORES)), trace=trace
    )
    return _assemble(LAST_RESULT.results)


def kernel(input, x_min, x_max, y_min, y_max):
    return _run({
        "input": np.asarray(input, dtype=np.float32),
        "x_min": np.asarray(x_min, dtype=np.float32),
        "x_max": np.asarray(x_max, dtype=np.float32),
        "y_min": np.asarray(y_min, dtype=np.float32),
        "y_max": np.asarray(y_max, dtype=np.float32),
    })
